# revision 19
# baseline (speedup 1.0000x reference)
"""Trainium2 Bass kernel for nn_ColdPrompt (dense_mlp).

Computes, for weight [B=256, P=4, D=768], W_spec [U=32, IN=3072, D=768],
b_spec [U=32, D=768]:
    prompt_emb    = weight.reshape(B, IN)                      # [256, 3072]
    task_specific = einsum('bi,uio->ubo', prompt_emb, W_spec) + b_spec[:,None,:]
                    -> reshape [U*B, D]                        # [8192, 768]
    mean_emb      = weight.mean(axis=1)                        # [256, 768]

Sharding: U sharded 4-users-per-core across 8 NeuronCores (expert-parallel),
prompt_emb replicated.  Per core the GEMM is [256,3072] @ [3072,768] x4 users,
computed on the PE with prompt_emb^T as the stationary operand (reused across
users / output columns) and W streamed from HBM exactly once.

Precision modes:
  "fp16"   - both operands rounded to fp16, fp32 PSUM accumulate (~3e-4 rel),
             full PE rate (1 cycle/row), same DMA volume as bf16
  "bf16"   - both operands rounded to bf16, fp32 PSUM accumulate (~2.4e-3 rel)
  "bf16x3" - hi/lo bf16 split of both operands, 3 accumulating passes
             (x_hi*W_hi + x_hi*W_lo + x_lo*W_hi), ~5e-6 rel, 3x PE work
"""

import os

import numpy as np
import ml_dtypes

import concourse.bass as bass
import concourse.bacc as bacc
import concourse.mybir as mybir
import concourse.tile as tile
from concourse.bass_utils import run_bass_kernel_spmd

BF16 = ml_dtypes.bfloat16

# Problem shapes (hardcoded per the contract).
B, P, D = 256, 4, 768
U = 32
IN = P * D            # 3072
NCORES = 8
UPC = U // NCORES     # users per core = 4
KT = IN // 128        # 24 contraction k-tiles
KB = 4                # k-tiles per W DMA block
NB = KT // KB         # 6 blocks
MT = B // 128         # 2 output row tiles
BPC = B // NCORES     # mean rows per core = 32
NSPLIT = (512, 256)   # matmul moving free dims (psum bank: <=512 fp32)
NT = len(NSPLIT)

MODE = os.environ.get("CC_KERNEL_MODE", "fp16")

_LAST_PERF = {}
_NC_CACHE = {}


def _build_nc(mode: str, reps: int = 1):
    """Build + compile the (SPMD, per-core) Bass module.

    reps>1 repeats the whole compute body (same inputs/outputs) so marginal
    per-rep time can be measured free of launch/transfer overhead."""
    nxt = 2 if mode == "bf16x3" else 1   # number of distinct x operands
    nwt = 2 if mode == "bf16x3" else 1   # number of distinct W streams
    mm_np = np.float16 if mode == "fp16" else BF16
    # (x_idx, w_idx) per accumulation pass, ordered so same-x passes are
    # adjacent (stationary-operand reuse on the PE).
    passes = [(0, 0), (0, 1), (1, 0)] if mode == "bf16x3" else [(0, 0)]

    nc = bacc.Bacc(
        "TRN2",
        target_bir_lowering=False,
        debug=False,
        enable_asserts=False,
        num_devices=NCORES,
    )

    f32 = mybir.dt.float32
    bf16 = mybir.dt.float16 if mode == "fp16" else mybir.dt.bfloat16

    xt_d = [
        nc.dram_tensor(f"xt{i}", [128, KT, B], bf16, kind="ExternalInput").ap()
        for i in range(nxt)
    ]
    w_d = [
        nc.dram_tensor(f"w{i}", [UPC, NB, 128, KB * D], bf16, kind="ExternalInput").ap()
        for i in range(nwt)
    ]
    # mean input: fp16 in fp16 mode (sum still fp32 on DVE, ~6e-5 rel err,
    # consistent with the task output's fp16 rounding), fp32 otherwise.
    wf_dt = bf16 if mode == "fp16" else f32
    wf_d = nc.dram_tensor("wf", [BPC, IN], wf_dt, kind="ExternalInput").ap()
    # bias in fp16 costs <=1e-5 abs error vs ~0.3 output scale in fp16 mode
    bias_dt = bf16 if mode == "fp16" else f32
    bias_d = nc.dram_tensor("bias", [128, UPC, D], bias_dt, kind="ExternalInput").ap()

    task_d = nc.dram_tensor(
        "task_out", [UPC, MT, 128, D], f32, kind="ExternalOutput"
    ).ap()
    mean_d = nc.dram_tensor("mean_out", [BPC, D], f32, kind="ExternalOutput").ap()

    with tile.TileContext(nc) as tc:
        with (
            tc.tile_pool(name="const", bufs=1) as cpool,
            tc.tile_pool(name="wstream", bufs=4) as wpool,
            tc.tile_pool(name="out", bufs=4) as opool,
            tc.tile_pool(name="meanbuf", bufs=2) as mpool,
            tc.tile_pool(name="psum", bufs=4, space=bass.MemorySpace.PSUM) as ppool,
        ):
            # x^T is loaded as 4 separate k-group tiles so the first matmuls
            # only gate on a small first group; the first W block is split in
            # half for the same reason. The sync engine issues DMAs in
            # program order, so emission order sets HBM arrival order:
            #   xt[k0:2] -> w(u0,k0:2) -> rest of xt interleaved with u0's W.
            XGROUPS = (2, 6, 8, 8)   # k-tiles per xt group tile
            xgoff = [sum(XGROUPS[:g]) for g in range(len(XGROUPS))]
            xt_sb = [[None] * len(XGROUPS) for _ in range(nxt)]

            def load_xt_group(g):
                for i in range(nxt):
                    t = cpool.tile(
                        [128, XGROUPS[g], B], bf16, tag=f"xt{i}g{g}",
                        name=f"xt{i}g{g}_sb",
                    )
                    nc.sync.dma_start(
                        t[:], xt_d[i][:, xgoff[g] : xgoff[g] + XGROUPS[g], :]
                    )
                    xt_sb[i][g] = t

            def xt_slice(xi, k, m):
                g = 0
                while k >= xgoff[g] + XGROUPS[g]:
                    g += 1
                return xt_sb[xi][g][:, k - xgoff[g], m * 128 : (m + 1) * 128]

            load_xt_group(0)
            bias_sb = cpool.tile([128, UPC, D], bias_dt, tag="bias")

            # Main per-user GEMM: accumulate over all k into 4 psum tiles
            # (2 row-tiles x 2 col-tiles), W streamed HBM->SBUF per k-block.
            for _rep in range(reps):
                for u in range(UPC):
                    first = _rep == 0 and u == 0
                    ps = [
                        [
                            ppool.tile([128, nw], f32, tag=f"ps{ni}", name="ps")
                            for ni, nw in enumerate(NSPLIT)
                        ]
                        for _ in range(MT)
                    ]
                    for kb in range(NB):
                        # sub-blocks of KB k-tiles per W DMA: the very first
                        # block is halved so the PE can start sooner
                        subs = [(0, 2), (2, 2)] if (first and kb == 0) else [(0, KB)]
                        w_sb = []  # [wi][sub] tiles
                        for wi in range(nwt):
                            tiles = []
                            for si, (ki0, nki) in enumerate(subs):
                                t = wpool.tile(
                                    [128, nki, D], bf16, tag=f"w{wi}",
                                    name=f"w{wi}_sb",
                                )
                                nc.sync.dma_start(
                                    t[:],
                                    w_d[wi][u, kb][:, ki0 * D : (ki0 + nki) * D],
                                )
                                tiles.append(t)
                            w_sb.append(tiles)
                        if first and kb == 0:
                            load_xt_group(1)
                        if first and kb == 1:
                            load_xt_group(2)
                            load_xt_group(3)
                        for ki in range(KB):
                            si = 0
                            while ki >= subs[si][0] + subs[si][1]:
                                si += 1
                            kis = ki - subs[si][0]
                            k = kb * KB + ki
                            for m in range(MT):
                                for pi, (xi, wi) in enumerate(passes):
                                    lhsT = xt_slice(xi, k, m)
                                    noff = 0
                                    for n, nw in enumerate(NSPLIT):
                                        nc.tensor.matmul(
                                            ps[m][n][:],
                                            lhsT,
                                            w_sb[wi][si][:, kis, noff : noff + nw],
                                            start=(k == 0 and pi == 0),
                                            stop=(
                                                k == KT - 1 and pi == len(passes) - 1
                                            ),
                                        )
                                        noff += nw
                    if first:
                        # bias lands behind user 0's W stream in the DMA
                        # queues but is ready before the first psum drain
                        nc.sync.dma_start(bias_sb[:], bias_d[:])
                    # Drain psum -> sbuf with bias add, then DMA out.
                    for m in range(MT):
                        ot = opool.tile([128, D], f32, tag="ot")
                        noff = 0
                        for n, nw in enumerate(NSPLIT):
                            sl = slice(noff, noff + nw)
                            nc.vector.tensor_add(
                                ot[:, sl], ps[m][n][:], bias_sb[:, u, sl]
                            )
                            noff += nw
                        nc.sync.dma_start(task_d[u, m], ot[:])

                    if u == 0:
                        # mean_emb (rows sharded across cores): emitted right
                        # after user 0 so its DMAs/DVE work fill the early
                        # bubble instead of extending the kernel tail.
                        wf_sb = mpool.tile([BPC, IN], wf_dt, tag="wf")
                        nc.sync.dma_start(wf_sb[:], wf_d[:])
                        t0 = mpool.tile([BPC, D], f32, tag="mt0")
                        t1 = mpool.tile([BPC, D], f32, tag="mt1")
                        mo = mpool.tile([BPC, D], f32, tag="mo")
                        nc.vector.tensor_add(
                            t0[:], wf_sb[:, 0:D], wf_sb[:, D : 2 * D]
                        )
                        nc.vector.tensor_add(
                            t1[:], wf_sb[:, 2 * D : 3 * D], wf_sb[:, 3 * D :]
                        )
                        nc.vector.tensor_add(t0[:], t0[:], t1[:])
                        nc.scalar.mul(mo[:], t0[:], 0.25)
                        nc.sync.dma_start(mean_d[:], mo[:])

    nc.compile()
    return nc


def _split_hi_lo(a):
    hi = a.astype(BF16)
    lo = (a - hi.astype(np.float32)).astype(BF16)
    return hi, lo


def prepare_in_maps(mode, weight, W_spec, b_spec):
    weight = np.asarray(weight, dtype=np.float32)
    W_spec = np.asarray(W_spec, dtype=np.float32)
    b_spec = np.asarray(b_spec, dtype=np.float32)

    x = weight.reshape(B, IN)
    # xt[p, k, b] = x[b, 128k + p]
    xt_f32 = np.ascontiguousarray(x.T).reshape(KT, 128, B).transpose(1, 0, 2)
    if mode == "bf16x3":
        xh, xl = _split_hi_lo(xt_f32)
        xts = [np.ascontiguousarray(xh), np.ascontiguousarray(xl)]
        Wh, Wl = _split_hi_lo(W_spec)
        Ws = [Wh, Wl]
    else:
        mm_np = np.float16 if mode == "fp16" else BF16
        xts = [np.ascontiguousarray(xt_f32.astype(mm_np))]
        Ws = [W_spec.astype(mm_np)]

    wf_full = weight.reshape(B, IN).astype(
        np.float16 if mode == "fp16" else np.float32
    )

    in_maps = []
    for c in range(NCORES):
        us = slice(c * UPC, (c + 1) * UPC)
        m = {}
        for i, xti in enumerate(xts):
            m[f"xt{i}"] = xti
        for i, Wi in enumerate(Ws):
            # w[u, kb, p, ki*D + d] = W[c*UPC+u, (KB*kb+ki)*128 + p, d]
            wc = (
                Wi[us]
                .reshape(UPC, NB, KB, 128, D)
                .transpose(0, 1, 3, 2, 4)
                .reshape(UPC, NB, 128, KB * D)
            )
            m[f"w{i}"] = np.ascontiguousarray(wc)
        m["bias"] = np.ascontiguousarray(
            np.broadcast_to(b_spec[us][None, :, :], (128, UPC, D)).astype(
                np.float16 if mode == "fp16" else np.float32
            )
        )
        m["wf"] = np.ascontiguousarray(wf_full[c * BPC : (c + 1) * BPC])
        in_maps.append(m)
    return in_maps


def kernel(weight, W_spec, b_spec):
    mode = MODE
    if mode not in _NC_CACHE:
        _NC_CACHE[mode] = _build_nc(mode)
    nc = _NC_CACHE[mode]
    in_maps = prepare_in_maps(mode, weight, W_spec, b_spec)

    res = run_bass_kernel_spmd(
        nc,
        in_maps,
        core_ids=list(range(NCORES)),
        trace=bool(int(os.environ.get("CC_KERNEL_TRACE", "0"))),
    )
    _LAST_PERF.clear()
    _LAST_PERF.update(
        exec_time_ns=res.exec_time_ns,
        mean_exec_time_ns=res.mean_exec_time_ns,
        max_exec_time_core_id=res.max_exec_time_core_id,
        trace=res.instructions_and_trace[1] if res.instructions_and_trace else None,
    )

    task = np.concatenate(
        [res.results[c]["task_out"].reshape(UPC * B, D) for c in range(NCORES)], axis=0
    )
    mean = np.concatenate(
        [res.results[c]["mean_out"] for c in range(NCORES)], axis=0
    )
    return task.astype(np.float32, copy=False), mean.astype(np.float32, copy=False)


# revision 20
# speedup vs baseline: 1.0875x; 1.0875x over previous
"""Trainium2 Bass kernel for nn_ColdPrompt (dense_mlp).

Computes, for weight [B=256, P=4, D=768], W_spec [U=32, IN=3072, D=768],
b_spec [U=32, D=768]:
    prompt_emb    = weight.reshape(B, IN)                      # [256, 3072]
    task_specific = einsum('bi,uio->ubo', prompt_emb, W_spec) + b_spec[:,None,:]
                    -> reshape [U*B, D]                        # [8192, 768]
    mean_emb      = weight.mean(axis=1)                        # [256, 768]

Sharding: U sharded 4-users-per-core across 8 NeuronCores (expert-parallel),
prompt_emb replicated.  Per core the GEMM is [256,3072] @ [3072,768] x4 users,
computed on the PE with prompt_emb^T as the stationary operand (reused across
users / output columns) and W streamed from HBM exactly once.

Precision modes:
  "fp16"   - both operands rounded to fp16, fp32 PSUM accumulate (~3e-4 rel),
             full PE rate (1 cycle/row), same DMA volume as bf16
  "bf16"   - both operands rounded to bf16, fp32 PSUM accumulate (~2.4e-3 rel)
  "bf16x3" - hi/lo bf16 split of both operands, 3 accumulating passes
             (x_hi*W_hi + x_hi*W_lo + x_lo*W_hi), ~5e-6 rel, 3x PE work
"""

import os

import numpy as np
import ml_dtypes

import concourse.bass as bass
import concourse.bacc as bacc
import concourse.mybir as mybir
import concourse.tile as tile
from concourse.bass_utils import run_bass_kernel_spmd

BF16 = ml_dtypes.bfloat16

# Problem shapes (hardcoded per the contract).
B, P, D = 256, 4, 768
U = 32
IN = P * D            # 3072
NCORES = 8
UPC = U // NCORES     # users per core = 4
KT = IN // 128        # 24 contraction k-tiles
KB = 4                # k-tiles per W DMA block
NB = KT // KB         # 6 blocks
MT = B // 128         # 2 output row tiles
BPC = B // NCORES     # mean rows per core = 32
NSPLIT = (512, 256)   # matmul moving free dims (psum bank: <=512 fp32)
NT = len(NSPLIT)

MODE = os.environ.get("CC_KERNEL_MODE", "fp16")

_LAST_PERF = {}
_NC_CACHE = {}


def _build_nc(mode: str, reps: int = 1):
    """Build + compile the (SPMD, per-core) Bass module.

    reps>1 repeats the whole compute body (same inputs/outputs) so marginal
    per-rep time can be measured free of launch/transfer overhead."""
    nxt = 2 if mode == "bf16x3" else 1   # number of distinct x operands
    nwt = 2 if mode == "bf16x3" else 1   # number of distinct W streams
    mm_np = np.float16 if mode == "fp16" else BF16
    # (x_idx, w_idx) per accumulation pass, ordered so same-x passes are
    # adjacent (stationary-operand reuse on the PE).
    passes = [(0, 0), (0, 1), (1, 0)] if mode == "bf16x3" else [(0, 0)]

    nc = bacc.Bacc(
        "TRN2",
        target_bir_lowering=False,
        debug=False,
        enable_asserts=False,
        num_devices=NCORES,
    )

    f32 = mybir.dt.float32
    bf16 = mybir.dt.float16 if mode == "fp16" else mybir.dt.bfloat16

    xt_d = [
        nc.dram_tensor(f"xt{i}", [128, KT, B], bf16, kind="ExternalInput").ap()
        for i in range(nxt)
    ]
    w_d = [
        nc.dram_tensor(f"w{i}", [UPC, NB, 128, KB * D], bf16, kind="ExternalInput").ap()
        for i in range(nwt)
    ]
    # mean input: fp16 in fp16 mode (sum still fp32 on DVE, ~6e-5 rel err,
    # consistent with the task output's fp16 rounding), fp32 otherwise.
    wf_dt = bf16 if mode == "fp16" else f32
    wf_d = nc.dram_tensor("wf", [BPC, IN], wf_dt, kind="ExternalInput").ap()
    # bias in fp16 costs <=1e-5 abs error vs ~0.3 output scale in fp16 mode
    bias_dt = bf16 if mode == "fp16" else f32
    bias_d = nc.dram_tensor("bias", [128, UPC, D], bias_dt, kind="ExternalInput").ap()

    task_d = nc.dram_tensor(
        "task_out", [UPC, MT, 128, D], f32, kind="ExternalOutput"
    ).ap()
    mean_d = nc.dram_tensor("mean_out", [BPC, D], f32, kind="ExternalOutput").ap()

    with tile.TileContext(nc) as tc:
        with (
            tc.tile_pool(name="const", bufs=1) as cpool,
            tc.tile_pool(name="wstream", bufs=8) as wpool,
            tc.tile_pool(name="out", bufs=4) as opool,
            tc.tile_pool(name="meanbuf", bufs=2) as mpool,
            tc.tile_pool(name="psum", bufs=4, space=bass.MemorySpace.PSUM) as ppool,
        ):
            # x^T is loaded as 4 separate k-group tiles so the first matmuls
            # only gate on a small first group; the first W block is split in
            # half for the same reason. The sync engine issues DMAs in
            # program order, so emission order sets HBM arrival order:
            #   xt[k0:2] -> w(u0,k0:2) -> rest of xt interleaved with u0's W.
            XGROUPS = (2, 6, 8, 8)   # k-tiles per xt group tile
            xgoff = [sum(XGROUPS[:g]) for g in range(len(XGROUPS))]
            xt_sb = [[None] * len(XGROUPS) for _ in range(nxt)]

            def load_xt_group(g):
                for i in range(nxt):
                    t = cpool.tile(
                        [128, XGROUPS[g], B], bf16, tag=f"xt{i}g{g}",
                        name=f"xt{i}g{g}_sb",
                    )
                    nc.sync.dma_start(
                        t[:], xt_d[i][:, xgoff[g] : xgoff[g] + XGROUPS[g], :]
                    )
                    xt_sb[i][g] = t

            def xt_slice(xi, k, m):
                g = 0
                while k >= xgoff[g] + XGROUPS[g]:
                    g += 1
                return xt_sb[xi][g][:, k - xgoff[g], m * 128 : (m + 1) * 128]

            load_xt_group(0)
            bias_sb = cpool.tile([128, UPC, D], bias_dt, tag="bias")

            # Main per-user GEMM: accumulate over all k into 4 psum tiles
            # (2 row-tiles x 2 col-tiles), W streamed HBM->SBUF per k-block.
            for _rep in range(reps):
                for u in range(UPC):
                    first = _rep == 0 and u == 0
                    ps = [
                        [
                            ppool.tile([128, nw], f32, tag=f"ps{ni}", name="ps")
                            for ni, nw in enumerate(NSPLIT)
                        ]
                        for _ in range(MT)
                    ]
                    for kb in range(NB):
                        # sub-blocks of KB k-tiles per W DMA: the very first
                        # block is halved so the PE can start sooner
                        subs = [(0, 2), (2, 2)] if (first and kb == 0) else [(0, KB)]
                        w_sb = []  # [wi][sub] tiles
                        for wi in range(nwt):
                            tiles = []
                            for si, (ki0, nki) in enumerate(subs):
                                t = wpool.tile(
                                    [128, nki, D], bf16, tag=f"w{wi}",
                                    name=f"w{wi}_sb",
                                )
                                nc.sync.dma_start(
                                    t[:],
                                    w_d[wi][u, kb][:, ki0 * D : (ki0 + nki) * D],
                                )
                                tiles.append(t)
                            w_sb.append(tiles)
                        if first and kb == 0:
                            load_xt_group(1)
                        if first and kb == 1:
                            load_xt_group(2)
                        if first and kb == 3:
                            load_xt_group(3)
                        for ki in range(KB):
                            si = 0
                            while ki >= subs[si][0] + subs[si][1]:
                                si += 1
                            kis = ki - subs[si][0]
                            k = kb * KB + ki
                            for m in range(MT):
                                for pi, (xi, wi) in enumerate(passes):
                                    lhsT = xt_slice(xi, k, m)
                                    noff = 0
                                    for n, nw in enumerate(NSPLIT):
                                        nc.tensor.matmul(
                                            ps[m][n][:],
                                            lhsT,
                                            w_sb[wi][si][:, kis, noff : noff + nw],
                                            start=(k == 0 and pi == 0),
                                            stop=(
                                                k == KT - 1 and pi == len(passes) - 1
                                            ),
                                        )
                                        noff += nw
                    if first:
                        # bias lands behind user 0's W stream in the DMA
                        # queues but is ready before the first psum drain
                        nc.sync.dma_start(bias_sb[:], bias_d[:])
                    # Drain psum -> sbuf with bias add; store each n-chunk
                    # as soon as its drain lands so DVE and store DMA pipeline.
                    for m in range(MT):
                        ot = opool.tile([128, D], f32, tag="ot")
                        noff = 0
                        for n, nw in enumerate(NSPLIT):
                            sl = slice(noff, noff + nw)
                            nc.vector.tensor_add(
                                ot[:, sl], ps[m][n][:], bias_sb[:, u, sl]
                            )
                            nc.sync.dma_start(task_d[u, m][:, sl], ot[:, sl])
                            noff += nw

                    if u == 0:
                        # mean_emb (rows sharded across cores): emitted right
                        # after user 0 so its DMAs/DVE work fill the early
                        # bubble instead of extending the kernel tail.
                        wf_sb = mpool.tile([BPC, IN], wf_dt, tag="wf")
                        nc.sync.dma_start(wf_sb[:], wf_d[:])
                        t0 = mpool.tile([BPC, D], f32, tag="mt0")
                        t1 = mpool.tile([BPC, D], f32, tag="mt1")
                        mo = mpool.tile([BPC, D], f32, tag="mo")
                        nc.vector.tensor_add(
                            t0[:], wf_sb[:, 0:D], wf_sb[:, D : 2 * D]
                        )
                        nc.vector.tensor_add(
                            t1[:], wf_sb[:, 2 * D : 3 * D], wf_sb[:, 3 * D :]
                        )
                        nc.vector.tensor_add(t0[:], t0[:], t1[:])
                        nc.scalar.mul(mo[:], t0[:], 0.25)
                        nc.sync.dma_start(mean_d[:], mo[:])

    nc.compile()
    return nc


def _split_hi_lo(a):
    hi = a.astype(BF16)
    lo = (a - hi.astype(np.float32)).astype(BF16)
    return hi, lo


def prepare_in_maps(mode, weight, W_spec, b_spec):
    weight = np.asarray(weight, dtype=np.float32)
    W_spec = np.asarray(W_spec, dtype=np.float32)
    b_spec = np.asarray(b_spec, dtype=np.float32)

    x = weight.reshape(B, IN)
    # xt[p, k, b] = x[b, 128k + p]
    xt_f32 = np.ascontiguousarray(x.T).reshape(KT, 128, B).transpose(1, 0, 2)
    if mode == "bf16x3":
        xh, xl = _split_hi_lo(xt_f32)
        xts = [np.ascontiguousarray(xh), np.ascontiguousarray(xl)]
        Wh, Wl = _split_hi_lo(W_spec)
        Ws = [Wh, Wl]
    else:
        mm_np = np.float16 if mode == "fp16" else BF16
        xts = [np.ascontiguousarray(xt_f32.astype(mm_np))]
        Ws = [W_spec.astype(mm_np)]

    wf_full = weight.reshape(B, IN).astype(
        np.float16 if mode == "fp16" else np.float32
    )

    in_maps = []
    for c in range(NCORES):
        us = slice(c * UPC, (c + 1) * UPC)
        m = {}
        for i, xti in enumerate(xts):
            m[f"xt{i}"] = xti
        for i, Wi in enumerate(Ws):
            # w[u, kb, p, ki*D + d] = W[c*UPC+u, (KB*kb+ki)*128 + p, d]
            wc = (
                Wi[us]
                .reshape(UPC, NB, KB, 128, D)
                .transpose(0, 1, 3, 2, 4)
                .reshape(UPC, NB, 128, KB * D)
            )
            m[f"w{i}"] = np.ascontiguousarray(wc)
        m["bias"] = np.ascontiguousarray(
            np.broadcast_to(b_spec[us][None, :, :], (128, UPC, D)).astype(
                np.float16 if mode == "fp16" else np.float32
            )
        )
        m["wf"] = np.ascontiguousarray(wf_full[c * BPC : (c + 1) * BPC])
        in_maps.append(m)
    return in_maps


def kernel(weight, W_spec, b_spec):
    mode = MODE
    if mode not in _NC_CACHE:
        _NC_CACHE[mode] = _build_nc(mode)
    nc = _NC_CACHE[mode]
    in_maps = prepare_in_maps(mode, weight, W_spec, b_spec)

    res = run_bass_kernel_spmd(
        nc,
        in_maps,
        core_ids=list(range(NCORES)),
        trace=bool(int(os.environ.get("CC_KERNEL_TRACE", "0"))),
    )
    _LAST_PERF.clear()
    _LAST_PERF.update(
        exec_time_ns=res.exec_time_ns,
        mean_exec_time_ns=res.mean_exec_time_ns,
        max_exec_time_core_id=res.max_exec_time_core_id,
        trace=res.instructions_and_trace[1] if res.instructions_and_trace else None,
    )

    task = np.concatenate(
        [res.results[c]["task_out"].reshape(UPC * B, D) for c in range(NCORES)], axis=0
    )
    mean = np.concatenate(
        [res.results[c]["mean_out"] for c in range(NCORES)], axis=0
    )
    return task.astype(np.float32, copy=False), mean.astype(np.float32, copy=False)


# revision 21
# speedup vs baseline: 1.4818x; 1.3626x over previous
"""Trainium2 Bass kernel for nn_ColdPrompt (dense_mlp).

Computes, for weight [B=256, P=4, D=768], W_spec [U=32, IN=3072, D=768],
b_spec [U=32, D=768]:
    prompt_emb    = weight.reshape(B, IN)                      # [256, 3072]
    task_specific = einsum('bi,uio->ubo', prompt_emb, W_spec) + b_spec[:,None,:]
                    -> reshape [U*B, D]                        # [8192, 768]
    mean_emb      = weight.mean(axis=1)                        # [256, 768]

Sharding: U sharded 4-users-per-core across 8 NeuronCores (expert-parallel),
prompt_emb replicated; mean_emb rows are sharded 32-per-core.  Per core the
GEMM is [256,3072] @ [3072,768] x4 users, computed on the PE with
prompt_emb^T as the stationary operand (a [128,128] tile is reused across
users and both output column tiles per load) and W streamed from HBM exactly
once as the moving operand into 512+256-wide fp32 PSUM accumulation groups
over 24 k-tiles.  Launch-edge scheduling: x^T is loaded as (2,6,8,8)-k-tile
group tiles and the first W block is halved so the first matmul gates on
~0.5 MB instead of ~6 MB; bias/wf/mean work is emitted inside user 0's
stream to fill early DMA slack instead of extending the tail; psum drains
(DVE tensor_add with bias) store each n-chunk as soon as it lands.
Steady-state per-core time measures ~58-63 us =~ the 78.6 TF/s PE streaming
roofline; the TimelineSim cost model puts a one-shot launch at ~81 us
(DMA-bound: 24.6 MB/core at 360 GB/s).

Precision modes:
  "fp16"   - both operands rounded to fp16, fp32 PSUM accumulate (~3e-4 rel),
             full PE rate (1 cycle/row), same DMA volume as bf16
  "bf16"   - both operands rounded to bf16, fp32 PSUM accumulate (~2.4e-3 rel)
  "bf16x3" - hi/lo bf16 split of both operands, 3 accumulating passes
             (x_hi*W_hi + x_hi*W_lo + x_lo*W_hi), ~5e-6 rel, 3x PE work
"""

import os

import numpy as np
import ml_dtypes

import concourse.bass as bass
import concourse.bacc as bacc
import concourse.mybir as mybir
import concourse.tile as tile
from concourse.bass_utils import run_bass_kernel_spmd

BF16 = ml_dtypes.bfloat16

# Problem shapes (hardcoded per the contract).
B, P, D = 256, 4, 768
U = 32
IN = P * D            # 3072
NCORES = 8
UPC = U // NCORES     # users per core = 4
KT = IN // 128        # 24 contraction k-tiles
KB = 4                # k-tiles per W DMA block
NB = KT // KB         # 6 blocks
MT = B // 128         # 2 output row tiles
BPC = B // NCORES     # mean rows per core = 32
NSPLIT = (512, 256)   # matmul moving free dims (psum bank: <=512 fp32)
NT = len(NSPLIT)

MODE = os.environ.get("CC_KERNEL_MODE", "fp16")

_LAST_PERF = {}
_NC_CACHE = {}


def _build_nc(mode: str, reps: int = 1):
    """Build + compile the (SPMD, per-core) Bass module.

    reps>1 repeats the whole compute body (same inputs/outputs) so marginal
    per-rep time can be measured free of launch/transfer overhead."""
    nxt = 2 if mode == "bf16x3" else 1   # number of distinct x operands
    nwt = 2 if mode == "bf16x3" else 1   # number of distinct W streams
    mm_np = np.float16 if mode == "fp16" else BF16
    # (x_idx, w_idx) per accumulation pass, ordered so same-x passes are
    # adjacent (stationary-operand reuse on the PE).
    passes = [(0, 0), (0, 1), (1, 0)] if mode == "bf16x3" else [(0, 0)]

    nc = bacc.Bacc(
        "TRN2",
        target_bir_lowering=False,
        debug=False,
        enable_asserts=False,
        num_devices=NCORES,
    )

    f32 = mybir.dt.float32
    bf16 = mybir.dt.float16 if mode == "fp16" else mybir.dt.bfloat16

    xt_d = [
        nc.dram_tensor(f"xt{i}", [128, KT, B], bf16, kind="ExternalInput").ap()
        for i in range(nxt)
    ]
    w_d = [
        nc.dram_tensor(f"w{i}", [UPC, NB, 128, KB * D], bf16, kind="ExternalInput").ap()
        for i in range(nwt)
    ]
    # mean input: fp16 in fp16 mode (sum still fp32 on DVE, ~6e-5 rel err,
    # consistent with the task output's fp16 rounding), fp32 otherwise.
    wf_dt = bf16 if mode == "fp16" else f32
    wf_d = nc.dram_tensor("wf", [BPC, IN], wf_dt, kind="ExternalInput").ap()
    # bias in fp16 costs <=1e-5 abs error vs ~0.3 output scale in fp16 mode
    bias_dt = bf16 if mode == "fp16" else f32
    bias_d = nc.dram_tensor("bias", [128, UPC, D], bias_dt, kind="ExternalInput").ap()

    task_d = nc.dram_tensor(
        "task_out", [UPC, MT, 128, D], f32, kind="ExternalOutput"
    ).ap()
    mean_d = nc.dram_tensor("mean_out", [BPC, D], f32, kind="ExternalOutput").ap()

    with tile.TileContext(nc) as tc:
        with (
            tc.tile_pool(name="const", bufs=1) as cpool,
            tc.tile_pool(name="wstream", bufs=8) as wpool,
            tc.tile_pool(name="out", bufs=4) as opool,
            tc.tile_pool(name="meanbuf", bufs=2) as mpool,
            tc.tile_pool(name="psum", bufs=4, space=bass.MemorySpace.PSUM) as ppool,
        ):
            # x^T is loaded as 4 separate k-group tiles so the first matmuls
            # only gate on a small first group; the first W block is split in
            # half for the same reason. The sync engine issues DMAs in
            # program order, so emission order sets HBM arrival order:
            #   xt[k0:2] -> w(u0,k0:2) -> rest of xt interleaved with u0's W.
            XGROUPS = (2, 6, 8, 8)   # k-tiles per xt group tile
            xgoff = [sum(XGROUPS[:g]) for g in range(len(XGROUPS))]
            xt_sb = [[None] * len(XGROUPS) for _ in range(nxt)]

            def load_xt_group(g):
                for i in range(nxt):
                    t = cpool.tile(
                        [128, XGROUPS[g], B], bf16, tag=f"xt{i}g{g}",
                        name=f"xt{i}g{g}_sb",
                    )
                    nc.sync.dma_start(
                        t[:], xt_d[i][:, xgoff[g] : xgoff[g] + XGROUPS[g], :]
                    )
                    xt_sb[i][g] = t

            def xt_slice(xi, k, m):
                g = 0
                while k >= xgoff[g] + XGROUPS[g]:
                    g += 1
                return xt_sb[xi][g][:, k - xgoff[g], m * 128 : (m + 1) * 128]

            load_xt_group(0)
            bias_sb = cpool.tile([128, UPC, D], bias_dt, tag="bias")

            # Main per-user GEMM: accumulate over all k into 4 psum tiles
            # (2 row-tiles x 2 col-tiles), W streamed HBM->SBUF per k-block.
            for _rep in range(reps):
                for u in range(UPC):
                    first = _rep == 0 and u == 0
                    ps = [
                        [
                            ppool.tile([128, nw], f32, tag=f"ps{ni}", name="ps")
                            for ni, nw in enumerate(NSPLIT)
                        ]
                        for _ in range(MT)
                    ]
                    for kb in range(NB):
                        # sub-blocks of KB k-tiles per W DMA: the very first
                        # block is halved so the PE can start sooner
                        subs = [(0, 2), (2, 2)] if (first and kb == 0) else [(0, KB)]
                        w_sb = []  # [wi][sub] tiles
                        for wi in range(nwt):
                            tiles = []
                            for si, (ki0, nki) in enumerate(subs):
                                t = wpool.tile(
                                    [128, nki, D], bf16, tag=f"w{wi}",
                                    name=f"w{wi}_sb",
                                )
                                nc.sync.dma_start(
                                    t[:],
                                    w_d[wi][u, kb][:, ki0 * D : (ki0 + nki) * D],
                                )
                                tiles.append(t)
                            w_sb.append(tiles)
                        if first and kb == 0:
                            load_xt_group(1)
                        if first and kb == 1:
                            load_xt_group(2)
                        if first and kb == 3:
                            load_xt_group(3)
                        for ki in range(KB):
                            si = 0
                            while ki >= subs[si][0] + subs[si][1]:
                                si += 1
                            kis = ki - subs[si][0]
                            k = kb * KB + ki
                            for m in range(MT):
                                for pi, (xi, wi) in enumerate(passes):
                                    lhsT = xt_slice(xi, k, m)
                                    noff = 0
                                    for n, nw in enumerate(NSPLIT):
                                        nc.tensor.matmul(
                                            ps[m][n][:],
                                            lhsT,
                                            w_sb[wi][si][:, kis, noff : noff + nw],
                                            start=(k == 0 and pi == 0),
                                            stop=(
                                                k == KT - 1 and pi == len(passes) - 1
                                            ),
                                        )
                                        noff += nw
                    if first:
                        # bias lands behind user 0's W stream in the DMA
                        # queues but is ready before the first psum drain
                        nc.sync.dma_start(bias_sb[:], bias_d[:])
                    # Drain psum -> sbuf with bias add; store each n-chunk
                    # as soon as its drain lands so DVE and store DMA pipeline.
                    for m in range(MT):
                        ot = opool.tile([128, D], f32, tag="ot")
                        noff = 0
                        for n, nw in enumerate(NSPLIT):
                            sl = slice(noff, noff + nw)
                            nc.vector.tensor_add(
                                ot[:, sl], ps[m][n][:], bias_sb[:, u, sl]
                            )
                            nc.sync.dma_start(task_d[u, m][:, sl], ot[:, sl])
                            noff += nw

                    if u == 0:
                        # mean_emb (rows sharded across cores): emitted right
                        # after user 0 so its DMAs/DVE work fill the early
                        # bubble instead of extending the kernel tail.
                        wf_sb = mpool.tile([BPC, IN], wf_dt, tag="wf")
                        nc.sync.dma_start(wf_sb[:], wf_d[:])
                        t0 = mpool.tile([BPC, D], f32, tag="mt0")
                        t1 = mpool.tile([BPC, D], f32, tag="mt1")
                        mo = mpool.tile([BPC, D], f32, tag="mo")
                        nc.vector.tensor_add(
                            t0[:], wf_sb[:, 0:D], wf_sb[:, D : 2 * D]
                        )
                        nc.vector.tensor_add(
                            t1[:], wf_sb[:, 2 * D : 3 * D], wf_sb[:, 3 * D :]
                        )
                        nc.vector.tensor_add(t0[:], t0[:], t1[:])
                        nc.scalar.mul(mo[:], t0[:], 0.25)
                        nc.sync.dma_start(mean_d[:], mo[:])

    nc.compile()
    return nc


def _split_hi_lo(a):
    hi = a.astype(BF16)
    lo = (a - hi.astype(np.float32)).astype(BF16)
    return hi, lo


def prepare_in_maps(mode, weight, W_spec, b_spec):
    weight = np.asarray(weight, dtype=np.float32)
    W_spec = np.asarray(W_spec, dtype=np.float32)
    b_spec = np.asarray(b_spec, dtype=np.float32)

    x = weight.reshape(B, IN)
    # xt[p, k, b] = x[b, 128k + p]
    xt_f32 = np.ascontiguousarray(x.T).reshape(KT, 128, B).transpose(1, 0, 2)
    if mode == "bf16x3":
        xh, xl = _split_hi_lo(xt_f32)
        xts = [np.ascontiguousarray(xh), np.ascontiguousarray(xl)]
        Wh, Wl = _split_hi_lo(W_spec)
        Ws = [Wh, Wl]
    else:
        mm_np = np.float16 if mode == "fp16" else BF16
        xts = [np.ascontiguousarray(xt_f32.astype(mm_np))]
        Ws = [W_spec.astype(mm_np)]

    wf_full = weight.reshape(B, IN).astype(
        np.float16 if mode == "fp16" else np.float32
    )

    in_maps = []
    for c in range(NCORES):
        us = slice(c * UPC, (c + 1) * UPC)
        m = {}
        for i, xti in enumerate(xts):
            m[f"xt{i}"] = xti
        for i, Wi in enumerate(Ws):
            # w[u, kb, p, ki*D + d] = W[c*UPC+u, (KB*kb+ki)*128 + p, d]
            wc = (
                Wi[us]
                .reshape(UPC, NB, KB, 128, D)
                .transpose(0, 1, 3, 2, 4)
                .reshape(UPC, NB, 128, KB * D)
            )
            m[f"w{i}"] = np.ascontiguousarray(wc)
        m["bias"] = np.ascontiguousarray(
            np.broadcast_to(b_spec[us][None, :, :], (128, UPC, D)).astype(
                np.float16 if mode == "fp16" else np.float32
            )
        )
        m["wf"] = np.ascontiguousarray(wf_full[c * BPC : (c + 1) * BPC])
        in_maps.append(m)
    return in_maps


def kernel(weight, W_spec, b_spec):
    mode = MODE
    if mode not in _NC_CACHE:
        _NC_CACHE[mode] = _build_nc(mode)
    nc = _NC_CACHE[mode]
    in_maps = prepare_in_maps(mode, weight, W_spec, b_spec)

    res = run_bass_kernel_spmd(
        nc,
        in_maps,
        core_ids=list(range(NCORES)),
        trace=bool(int(os.environ.get("CC_KERNEL_TRACE", "0"))),
    )
    _LAST_PERF.clear()
    _LAST_PERF.update(
        exec_time_ns=res.exec_time_ns,
        mean_exec_time_ns=res.mean_exec_time_ns,
        max_exec_time_core_id=res.max_exec_time_core_id,
        trace=res.instructions_and_trace[1] if res.instructions_and_trace else None,
    )

    task = np.concatenate(
        [res.results[c]["task_out"].reshape(UPC * B, D) for c in range(NCORES)], axis=0
    )
    mean = np.concatenate(
        [res.results[c]["mean_out"] for c in range(NCORES)], axis=0
    )
    return task.astype(np.float32, copy=False), mean.astype(np.float32, copy=False)
